# revision 12
# baseline (speedup 1.0000x reference)
"""Bidirectional attention (softmax(+logits) and softmax(-logits) branches)
on 8 Trainium2 NeuronCores.

Sharding: batch x head-group. Core c handles batch c//4 and heads
4*(c%4) .. 4*(c%4)+3. Each core computes its heads' Q/K/V projections,
both softmax branches, and a partial output projection (row-shard of Wo);
the host sums the 4 partials per batch and transposes.

All matmuls run in bf16 (fp32 matmul is 4x slower on the PE); PSUM
accumulation is fp32. The softmax uses unnormalized exp (logit range is
~N(0,1), no max-subtraction needed): P = exp(+/-logits) via wide ACT ops,
Z comes free from a ones-column appended to V, and the 1/Z normalization
uses reciprocal_approx_fast + a step-0-free-dim DMA broadcast.

PE throughput tricks (all HW-measured):
- head-pair col-tiled projections (even head -> psum[0:64], odd ->
  psum[64:128]) with 4 interleaved chunk accumulation chains;
- Q^T/K^T kept in both base-0 and base-64 copies so consecutive K=64
  logit matmuls alternate PE row-groups (~2x concurrency);
- featT/outproj accumulation chains interleaved in pairs sharing lhsT
  (single accumulation chains serialize fill/drain and are ~3x slower).

Host-side prep folds the 1/sqrt(d) scale into Wq, and bv@Wo+bo into a
host-side bias (exact because softmax rows sum to 1).
"""

import os
import sys

for _p in ("/opt/trn_rl_repo",):
    if _p not in sys.path:
        sys.path.insert(0, _p)

import numpy as np
import ml_dtypes

import concourse.bass as bass
import concourse.tile as tile
from concourse import bacc, mybir
from concourse import bass_utils

BF16 = ml_dtypes.bfloat16

B, S, D = 2, 2048, 1024
NUM_HEADS, HEAD_DIM = 16, 64
N_CORES = 8
GROUPS = 4                      # head groups (one per core within a batch)
H = NUM_HEADS // GROUPS         # heads per core = 4
DH = H * HEAD_DIM               # per-core head dims = 256
QCH = 512                       # q-chunk (matmul moving free dim)
NCH = S // QCH                  # 4 q-chunks
SK = S // 128                   # 16 s_k tiles
VSTRIDE = H * 65                # V' row stride: 4 heads x (64 + ones col)

f32 = mybir.dt.float32
bf16 = mybir.dt.bfloat16
EXP = mybir.ActivationFunctionType.Exp
IDENT = mybir.ActivationFunctionType.Identity

_CACHE = {}


def _emit(tc):
    nc = tc.nc
    tokT = nc.dram_tensor("tokT", [D, S], bf16, kind="ExternalInput").ap()
    wq = nc.dram_tensor("wq", [128, 8 * DH], bf16, kind="ExternalInput").ap()
    wk = nc.dram_tensor("wk", [128, 8 * DH], bf16, kind="ExternalInput").ap()
    wv = nc.dram_tensor("wv", [128, 8 * DH], bf16, kind="ExternalInput").ap()
    wo = nc.dram_tensor("wo", [128, 2 * D], bf16, kind="ExternalInput").ap()
    bqk = nc.dram_tensor("bqk", [128, 4], f32, kind="ExternalInput").ap()
    outs = [
        nc.dram_tensor("outT_neg", [D, S], f32, kind="ExternalOutput").ap(),
        nc.dram_tensor("outT_pos", [D, S], f32, kind="ExternalOutput").ap(),
    ]

    import contextlib

    with contextlib.ExitStack() as ctx:
        wp = ctx.enter_context(tc.tile_pool(name="wp", bufs=1))
        act = ctx.enter_context(tc.tile_pool(name="act", bufs=1))
        ppool = ctx.enter_context(tc.tile_pool(name="pp", bufs=2))
        fpool = ctx.enter_context(tc.tile_pool(name="fp", bufs=4))
        zpool = ctx.enter_context(tc.tile_pool(name="zp", bufs=4))
        opool = ctx.enter_context(tc.tile_pool(name="op", bufs=8))

        # ---- weight / bias / token loads -------------------------------
        wq_sb = wp.tile([128, 8 * DH], bf16, tag="wq")
        wk_sb = wp.tile([128, 8 * DH], bf16, tag="wk")
        wv_sb = wp.tile([128, 8 * DH], bf16, tag="wv")
        wo_sb = wp.tile([128, 2 * D], bf16, tag="wo")
        bqk_sb = wp.tile([128, 4], f32, tag="bqk")
        nc.sync.dma_start(wq_sb[:], wq)
        nc.sync.dma_start(wk_sb[:], wk)
        nc.sync.dma_start(wv_sb[:], wv)
        nc.sync.dma_start(wo_sb[:], wo)
        nc.sync.dma_start(bqk_sb[:], bqk)

        tok = []
        for t in range(8):
            tt = act.tile([128, S], bf16, tag=f"tok{t}")
            half = S // 2
            for hh in range(2):
                nc.sync.dma_start(tt[:, hh * half:(hh + 1) * half],
                                  tokT[t * 128:(t + 1) * 128, hh * half:(hh + 1) * half])
            tok.append(tt)

        # head-pair tiles: even head in partitions 0:64, odd in 64:128;
        # *_alt has the two halves swapped (so every head exists at both
        # partition bases -- lets logit matmuls alternate PE row groups)
        qt_pair = [act.tile([128, S], bf16, tag=f"qp{j}", name=f"qp{j}") for j in range(2)]
        kt_pair = [act.tile([128, S], bf16, tag=f"kp{j}", name=f"kp{j}") for j in range(2)]
        qt_alt = [act.tile([128, S], bf16, tag=f"qa{j}", name=f"qa{j}") for j in range(2)]
        kt_alt = [act.tile([128, S], bf16, tag=f"ka{j}", name=f"ka{j}") for j in range(2)]
        SKQ = SK // 4
        vp_q = [act.tile([128, SKQ * VSTRIDE], bf16, tag=f"vp{i}", name=f"vp{i}")
                for i in range(4)]

        def vap_for(st, h):
            q, r = divmod(st, SKQ)
            off = r * VSTRIDE + h * 65
            return vp_q[q][:, off:off + 65]

        # ---- phase 1: projections --------------------------------------
        with tc.tile_pool(name="pps", bufs=1, space="PSUM") as pps:
            def qk_proj(j):
                # head-pair col-tiled, 4 interleaved chunk chains
                for w_sb, pair, alt, bcol0 in (
                    (wq_sb, qt_pair, qt_alt, 0),
                    (wk_sb, kt_pair, kt_alt, 2),
                ):
                    ps = [pps.tile([128, QCH], f32, tag=f"pj{cc}", name=f"pj{cc}")
                          for cc in range(NCH)]
                    for t in range(8):
                        lo = t * DH + (2 * j) * 64
                        hi = t * DH + (2 * j + 1) * 64
                        for cc in range(NCH):
                            rhs = tok[t][:, cc * QCH:(cc + 1) * QCH]
                            nc.tensor.matmul(ps[cc][0:64, :], w_sb[:, lo:lo + 64],
                                             rhs, start=(t == 0), stop=(t == 7))
                            nc.tensor.matmul(ps[cc][64:128, :], w_sb[:, hi:hi + 64],
                                             rhs, start=(t == 0), stop=(t == 7))
                    for cc in range(NCH):
                        nc.scalar.activation(
                            pair[j][:, cc * QCH:(cc + 1) * QCH], ps[cc][:],
                            IDENT, bias=bqk_sb[:, bcol0 + j:bcol0 + j + 1],
                        )
                    nc.sync.dma_start(alt[j][64:128, :], pair[j][0:64, :])
                    nc.sync.dma_start(alt[j][0:64, :], pair[j][64:128, :])

            def v_proj():
                # 2 interleaved s-tile chains, strided into V' (ones cols
                # memset per quarter so featT deps stay local)
                for q in range(4):
                    ones_ap = vp_q[q].rearrange(
                        "p (s h x) -> p (s h) x", s=SKQ, h=H)[:, :, 64:65]
                    nc.gpsimd.memset(ones_ap, 1.0)
                for sp in range(SK // 2):
                    psv = [pps.tile([128, DH], f32, tag=f"pv{i}", name=f"pv{i}", bufs=2)
                           for i in range(2)]
                    for t in range(8):
                        for i in range(2):
                            st = sp * 2 + i
                            nc.tensor.matmul(
                                psv[i][:],
                                tok[t][:, st * 128:(st + 1) * 128],
                                wv_sb[:, t * DH:(t + 1) * DH],
                                start=(t == 0), stop=(t == 7),
                            )
                    for i in range(2):
                        st = sp * 2 + i
                        q, r = divmod(st, SKQ)
                        dst = vp_q[q][:, r * VSTRIDE:(r + 1) * VSTRIDE]
                        dst3 = dst.rearrange("p (h x) -> p h x", h=H)[:, :, 0:64]
                        src3 = psv[i].rearrange("p (h x) -> p h x", h=H)
                        nc.vector.tensor_copy(dst3, src3)

            qk_proj(0)
            v_proj()
            qk_proj(1)

        # ---- phase 2: attention + output projection --------------------
        GW = 2           # s_k-tiles per exp group (FD = GW*QCH = 1024)
        NG = SK // GW    # 8 groups per (head, chunk)
        with (
            tc.tile_pool(name="lgp", bufs=2, space="PSUM") as lgp,
            tc.tile_pool(name="ftp", bufs=2, space="PSUM") as ftp,
            tc.tile_pool(name="otp", bufs=1, space="PSUM") as otp,
        ):
            def emit_outproj(fsbx, c_idx, dts):
                for dt in dts:
                    ops = otp.tile([128, 2 * QCH], f32, tag="ot", name="ops")
                    for p in range(2):
                        lhs = wo_sb[:, p * D + dt * 128:p * D + (dt + 1) * 128]
                        for br in range(2):
                            nc.tensor.matmul(
                                ops[:, br * QCH:(br + 1) * QCH],
                                lhs,
                                fsbx[p][br][:],
                                start=(p == 0), stop=(p == 1),
                            )
                    osb = opool.tile([128, 2 * QCH], f32, tag="os", name="osb")
                    for br in range(2):
                        nc.vector.tensor_copy(osb[:, br * QCH:(br + 1) * QCH],
                                              ops[:, br * QCH:(br + 1) * QCH])
                        nc.sync.dma_start(
                            outs[br][dt * 128:(dt + 1) * 128,
                                     c_idx * QCH:(c_idx + 1) * QCH],
                            osb[:, br * QCH:(br + 1) * QCH],
                        )

            fsb_prev = None
            for c in range(NCH):
                fsb = [[None, None], [None, None]]  # [pair][branch]
                for p in range(2):
                    for br in range(2):
                        fsb[p][br] = fpool.tile([128, QCH], bf16, tag="fsb",
                                                bufs=8, name=f"fsb{p}{br}")
                for h in range(H):
                    j, par = h // 2, h % 2
                    # base-0 and base-64 views of this head's Q^T / K^T
                    k_lo = kt_pair[j] if par == 0 else kt_alt[j]
                    k_hi = kt_alt[j] if par == 0 else kt_pair[j]
                    q_lo = qt_pair[j] if par == 0 else qt_alt[j]
                    q_hi = qt_alt[j] if par == 0 else qt_pair[j]
                    ft = [ftp.tile([65, QCH], f32, tag="ft", name=f"ft{i}")
                          for i in range(2)]
                    for g in range(NG):
                        lg = lgp.tile([128, GW * QCH], f32, tag="lg")
                        for t2 in range(GW):
                            st = g * GW + t2
                            if st % 2 == 0:
                                nc.tensor.matmul(
                                    lg[:, t2 * QCH:(t2 + 1) * QCH],
                                    k_lo[0:64, st * 128:(st + 1) * 128],
                                    q_lo[0:64, c * QCH:(c + 1) * QCH],
                                    start=True, stop=True)
                            else:
                                nc.tensor.matmul(
                                    lg[:, t2 * QCH:(t2 + 1) * QCH],
                                    k_hi[64:128, st * 128:(st + 1) * 128],
                                    q_hi[64:128, c * QCH:(c + 1) * QCH],
                                    start=True, stop=True)
                        pw = [ppool.tile([128, GW * QCH], bf16, tag=f"pw{i}",
                                         name=f"pw{i}", bufs=3) for i in range(2)]
                        nc.scalar.activation(pw[0][:], lg[:], EXP, scale=-1.0)
                        nc.scalar.activation(pw[1][:], lg[:], EXP)
                        for t2 in range(GW):
                            st = g * GW + t2
                            vap = vap_for(st, h)
                            for br in range(2):
                                nc.tensor.matmul(
                                    ft[br][:],
                                    vap,
                                    pw[br][:, t2 * QCH:(t2 + 1) * QCH],
                                    start=(st == 0),
                                    stop=(st == SK - 1),
                                )
                    # interleave the previous chunk's output projection into
                    # this chunk's ACT-bound stretch (2 dout-tiles per head);
                    # emitted BEFORE the Z-path so its PSUM evac isn't queued
                    # behind the Z-path on the in-order DVE
                    if fsb_prev is not None:
                        emit_outproj(fsb_prev, c - 1, range(2 * h, 2 * h + 2))
                    # normalize: feat / Z, Z = row 64.  Stage-major across
                    # branches so the DVE never idles waiting on the
                    # broadcast DMA mid-chain.  The Z row goes PSUM@p64 ->
                    # SBUF@p64 (plain copy; engines cannot shift partitions),
                    # broadcast-DMA (gpsimd queue) to [64,:]@p0, reciprocal,
                    # multiply.
                    ftc = [zpool.tile([65, QCH], f32, tag=f"ftc{i}", name=f"ftc{i}")
                           for i in range(2)]
                    zraw = [zpool.tile([64, QCH], f32, tag=f"zraw{i}", name=f"zraw{i}")
                            for i in range(2)]
                    zbt = [zpool.tile([64, QCH], f32, tag=f"zb{i}", name=f"zb{i}")
                           for i in range(2)]
                    for br in range(2):
                        nc.vector.tensor_copy(ftc[br][:], ft[br][:])
                        nc.gpsimd.dma_start(
                            zraw[br][:],
                            ftc[br][64:65, :].rearrange("p (o f) -> p o f", o=1)
                            .to_broadcast([1, 64, QCH]),
                        )
                    for br in range(2):
                        nc.vector.reciprocal_approx_fast(zbt[br][:], zraw[br][:])
                    for br in range(2):
                        if par == 0:
                            nc.vector.tensor_mul(
                                fsb[j][br][0:64, :], ftc[br][0:64, :], zbt[br][:])
                        else:
                            tmp = zpool.tile([64, QCH], bf16, tag=f"tmp{br}",
                                             name=f"tmp{br}")
                            nc.vector.tensor_mul(tmp[:], ftc[br][0:64, :], zbt[br][:])
                            nc.gpsimd.dma_start(fsb[j][br][64:128, :], tmp[:])
                fsb_prev = fsb
            emit_outproj(fsb_prev, NCH - 1, range(8))


def _build():
    if "nc" in _CACHE:
        return _CACHE["nc"]
    nc = bacc.Bacc("TRN2", target_bir_lowering=False, debug=False,
                   num_devices=N_CORES)
    with tile.TileContext(nc) as tc:
        _emit(tc)
    nc.compile()
    _CACHE["nc"] = nc
    return nc


def _prep_core_inputs(tokens, Wq, bq, Wk, bk, Wv, bv, Wo, bo):
    """Host-side marshaling: slice per core, transpose tokens, cast bf16."""
    scale = 1.0 / np.sqrt(HEAD_DIM)
    per_batch_tokT = [
        np.ascontiguousarray(tokens[b].T).astype(BF16) for b in range(B)
    ]
    in_maps = []
    for core in range(N_CORES):
        b, g = divmod(core, GROUPS)
        cols = slice(g * DH, (g + 1) * DH)
        # weights as [128, 8*DH]: din-tile t at column block t
        wq_s = (Wq[:, cols] * scale).astype(BF16).reshape(8, 128, DH)
        wq_s = np.ascontiguousarray(wq_s.transpose(1, 0, 2)).reshape(128, 8 * DH)
        wk_s = Wk[:, cols].astype(BF16).reshape(8, 128, DH)
        wk_s = np.ascontiguousarray(wk_s.transpose(1, 0, 2)).reshape(128, 8 * DH)
        wv_s = Wv[:, cols].astype(BF16).reshape(8, 128, DH)
        wv_s = np.ascontiguousarray(wv_s.transpose(1, 0, 2)).reshape(128, 8 * DH)
        # Wo rows for this group, pair p at column block p
        wo_s = Wo[cols, :].astype(BF16).reshape(2, 128, D)
        wo_s = np.ascontiguousarray(wo_s.transpose(1, 0, 2)).reshape(128, 2 * D)
        # biases: column j = q-pair j (rows 0:64 even head, 64:128 odd),
        # column 2+j = k-pair j
        bqk_s = np.zeros((128, 4), np.float32)
        for j in range(2):
            bqk_s[0:64, j] = bq[g * DH + (2 * j) * 64:g * DH + (2 * j + 1) * 64] * scale
            bqk_s[64:128, j] = bq[g * DH + (2 * j + 1) * 64:g * DH + (2 * j + 2) * 64] * scale
            bqk_s[0:64, 2 + j] = bk[g * DH + (2 * j) * 64:g * DH + (2 * j + 1) * 64]
            bqk_s[64:128, 2 + j] = bk[g * DH + (2 * j + 1) * 64:g * DH + (2 * j + 2) * 64]
        in_maps.append({
            "tokT": per_batch_tokT[b],
            "wq": wq_s, "wk": wk_s, "wv": wv_s, "wo": wo_s,
            "bqk": bqk_s,
        })
    return in_maps


def kernel(tokens, Wq, bq, Wk, bk, Wv, bv, Wo, bo):
    tokens = np.asarray(tokens, np.float32)
    Wq = np.asarray(Wq, np.float32); bq = np.asarray(bq, np.float32)
    Wk = np.asarray(Wk, np.float32); bk = np.asarray(bk, np.float32)
    Wv = np.asarray(Wv, np.float32); bv = np.asarray(bv, np.float32)
    Wo = np.asarray(Wo, np.float32); bo = np.asarray(bo, np.float32)

    nc = _build()
    in_maps = _prep_core_inputs(tokens, Wq, bq, Wk, bk, Wv, bv, Wo, bo)
    res = bass_utils.run_bass_kernel_spmd(
        nc, in_maps, core_ids=list(range(N_CORES)))
    _CACHE["last_result"] = res

    bo_eff = (bv.astype(np.float64) @ Wo.astype(np.float64)
              + bo.astype(np.float64)).astype(np.float32)

    out = []
    for name in ("outT_neg", "outT_pos"):
        full = np.empty((B, S, D), np.float32)
        for b in range(B):
            acc = res.results[b * GROUPS][name].copy()
            for g in range(1, GROUPS):
                acc += res.results[b * GROUPS + g][name]
            full[b] = acc.T
        full += bo_eff
        out.append(full)
    return tuple(out)


# revision 13
# speedup vs baseline: 1.0031x; 1.0031x over previous
"""Bidirectional attention (softmax(+logits) and softmax(-logits) branches)
on 8 Trainium2 NeuronCores.

Sharding: batch x head-group. Core c handles batch c//4 and heads
4*(c%4) .. 4*(c%4)+3. Each core computes its heads' Q/K/V projections,
both softmax branches, and a partial output projection (row-shard of Wo);
the host sums the 4 partials per batch and transposes.

All matmuls run in bf16 (fp32 matmul is 4x slower on the PE); PSUM
accumulation is fp32. The softmax uses unnormalized exp (logit range is
~N(0,1), no max-subtraction needed): P = exp(+/-logits) via wide ACT ops,
Z comes free from a ones-column appended to V, and the 1/Z normalization
uses reciprocal_approx_fast + a step-0-free-dim DMA broadcast.

PE throughput tricks (all HW-measured):
- head-pair col-tiled projections (even head -> psum[0:64], odd ->
  psum[64:128]) with 4 interleaved chunk accumulation chains;
- Q^T/K^T kept in both base-0 and base-64 copies so consecutive K=64
  logit matmuls alternate PE row-groups (~2x concurrency);
- featT/outproj accumulation chains interleaved in pairs sharing lhsT
  (single accumulation chains serialize fill/drain and are ~3x slower).

Host-side prep folds the 1/sqrt(d) scale into Wq, and bv@Wo+bo into a
host-side bias (exact because softmax rows sum to 1).
"""

import os
import sys

for _p in ("/opt/trn_rl_repo",):
    if _p not in sys.path:
        sys.path.insert(0, _p)

import numpy as np
import ml_dtypes

import concourse.bass as bass
import concourse.tile as tile
from concourse import bacc, mybir
from concourse import bass_utils

BF16 = ml_dtypes.bfloat16

B, S, D = 2, 2048, 1024
NUM_HEADS, HEAD_DIM = 16, 64
N_CORES = 8
GROUPS = 4                      # head groups (one per core within a batch)
H = NUM_HEADS // GROUPS         # heads per core = 4
DH = H * HEAD_DIM               # per-core head dims = 256
QCH = 512                       # q-chunk (matmul moving free dim)
NCH = S // QCH                  # 4 q-chunks
SK = S // 128                   # 16 s_k tiles
VSTRIDE = H * 65                # V' row stride: 4 heads x (64 + ones col)

f32 = mybir.dt.float32
bf16 = mybir.dt.bfloat16
EXP = mybir.ActivationFunctionType.Exp
IDENT = mybir.ActivationFunctionType.Identity

_CACHE = {}


def _emit(tc):
    nc = tc.nc
    tokT = nc.dram_tensor("tokT", [D, S], bf16, kind="ExternalInput").ap()
    wq = nc.dram_tensor("wq", [128, 8 * DH], bf16, kind="ExternalInput").ap()
    wk = nc.dram_tensor("wk", [128, 8 * DH], bf16, kind="ExternalInput").ap()
    wv = nc.dram_tensor("wv", [128, 8 * DH], bf16, kind="ExternalInput").ap()
    wo = nc.dram_tensor("wo", [128, 2 * D], bf16, kind="ExternalInput").ap()
    bqk = nc.dram_tensor("bqk", [128, 4], f32, kind="ExternalInput").ap()
    outs = [
        nc.dram_tensor("outT_neg", [D, S], f32, kind="ExternalOutput").ap(),
        nc.dram_tensor("outT_pos", [D, S], f32, kind="ExternalOutput").ap(),
    ]

    import contextlib

    with contextlib.ExitStack() as ctx:
        wp = ctx.enter_context(tc.tile_pool(name="wp", bufs=1))
        act = ctx.enter_context(tc.tile_pool(name="act", bufs=1))
        ppool = ctx.enter_context(tc.tile_pool(name="pp", bufs=2))
        fpool = ctx.enter_context(tc.tile_pool(name="fp", bufs=4))
        zpool = ctx.enter_context(tc.tile_pool(name="zp", bufs=4))
        opool = ctx.enter_context(tc.tile_pool(name="op", bufs=8))

        # ---- weight / bias / token loads -------------------------------
        wq_sb = wp.tile([128, 8 * DH], bf16, tag="wq")
        wk_sb = wp.tile([128, 8 * DH], bf16, tag="wk")
        wv_sb = wp.tile([128, 8 * DH], bf16, tag="wv")
        wo_sb = wp.tile([128, 2 * D], bf16, tag="wo")
        bqk_sb = wp.tile([128, 4], f32, tag="bqk")
        nc.sync.dma_start(wq_sb[:], wq)
        nc.sync.dma_start(wk_sb[:], wk)
        nc.sync.dma_start(wv_sb[:], wv)
        nc.sync.dma_start(wo_sb[:], wo)
        nc.sync.dma_start(bqk_sb[:], bqk)

        tok = []
        for t in range(8):
            tt = act.tile([128, S], bf16, tag=f"tok{t}")
            half = S // 2
            for hh in range(2):
                nc.sync.dma_start(tt[:, hh * half:(hh + 1) * half],
                                  tokT[t * 128:(t + 1) * 128, hh * half:(hh + 1) * half])
            tok.append(tt)

        # head-pair tiles: even head in partitions 0:64, odd in 64:128;
        # *_alt has the two halves swapped (so every head exists at both
        # partition bases -- lets logit matmuls alternate PE row groups)
        qt_pair = [act.tile([128, S], bf16, tag=f"qp{j}", name=f"qp{j}") for j in range(2)]
        kt_pair = [act.tile([128, S], bf16, tag=f"kp{j}", name=f"kp{j}") for j in range(2)]
        qt_alt = [act.tile([128, S], bf16, tag=f"qa{j}", name=f"qa{j}") for j in range(2)]
        kt_alt = [act.tile([128, S], bf16, tag=f"ka{j}", name=f"ka{j}") for j in range(2)]
        SKQ = SK // 4
        vp_q = [act.tile([128, SKQ * VSTRIDE], bf16, tag=f"vp{i}", name=f"vp{i}")
                for i in range(4)]

        def vap_for(st, h):
            q, r = divmod(st, SKQ)
            off = r * VSTRIDE + h * 65
            return vp_q[q][:, off:off + 65]

        # ---- phase 1: projections --------------------------------------
        with tc.tile_pool(name="pps", bufs=1, space="PSUM") as pps:
            def qk_proj(j):
                # head-pair col-tiled, 4 interleaved chunk chains
                for w_sb, pair, alt, bcol0 in (
                    (wq_sb, qt_pair, qt_alt, 0),
                    (wk_sb, kt_pair, kt_alt, 2),
                ):
                    ps = [pps.tile([128, QCH], f32, tag=f"pj{cc}", name=f"pj{cc}")
                          for cc in range(NCH)]
                    for t in range(8):
                        lo = t * DH + (2 * j) * 64
                        hi = t * DH + (2 * j + 1) * 64
                        for cc in range(NCH):
                            rhs = tok[t][:, cc * QCH:(cc + 1) * QCH]
                            nc.tensor.matmul(ps[cc][0:64, :], w_sb[:, lo:lo + 64],
                                             rhs, start=(t == 0), stop=(t == 7))
                            nc.tensor.matmul(ps[cc][64:128, :], w_sb[:, hi:hi + 64],
                                             rhs, start=(t == 0), stop=(t == 7))
                    for cc in range(NCH):
                        nc.scalar.activation(
                            pair[j][:, cc * QCH:(cc + 1) * QCH], ps[cc][:],
                            IDENT, bias=bqk_sb[:, bcol0 + j:bcol0 + j + 1],
                        )
                    nc.sync.dma_start(alt[j][64:128, :], pair[j][0:64, :])
                    nc.sync.dma_start(alt[j][0:64, :], pair[j][64:128, :])

            def v_proj():
                # 2 interleaved s-tile chains, strided into V' (ones cols
                # memset per quarter so featT deps stay local)
                for q in range(4):
                    ones_ap = vp_q[q].rearrange(
                        "p (s h x) -> p (s h) x", s=SKQ, h=H)[:, :, 64:65]
                    nc.gpsimd.memset(ones_ap, 1.0)
                for sp in range(SK // 2):
                    psv = [pps.tile([128, DH], f32, tag=f"pv{i}", name=f"pv{i}", bufs=2)
                           for i in range(2)]
                    for t in range(8):
                        for i in range(2):
                            st = sp * 2 + i
                            nc.tensor.matmul(
                                psv[i][:],
                                tok[t][:, st * 128:(st + 1) * 128],
                                wv_sb[:, t * DH:(t + 1) * DH],
                                start=(t == 0), stop=(t == 7),
                            )
                    for i in range(2):
                        st = sp * 2 + i
                        q, r = divmod(st, SKQ)
                        dst = vp_q[q][:, r * VSTRIDE:(r + 1) * VSTRIDE]
                        dst3 = dst.rearrange("p (h x) -> p h x", h=H)[:, :, 0:64]
                        src3 = psv[i].rearrange("p (h x) -> p h x", h=H)
                        nc.vector.tensor_copy(dst3, src3)

            qk_proj(0)
            v_proj()
            qk_proj(1)

        # ---- phase 2: attention + output projection --------------------
        GW = 2           # s_k-tiles per exp group (FD = GW*QCH = 1024)
        NG = SK // GW    # 8 groups per (head, chunk)
        with (
            tc.tile_pool(name="lgp", bufs=2, space="PSUM") as lgp,
            tc.tile_pool(name="ftp", bufs=2, space="PSUM") as ftp,
            tc.tile_pool(name="otp", bufs=1, space="PSUM") as otp,
        ):
            def emit_outproj(fsbx, c_idx, dts):
                for dt in dts:
                    ops = otp.tile([128, 2 * QCH], f32, tag="ot", name="ops")
                    for p in range(2):
                        lhs = wo_sb[:, p * D + dt * 128:p * D + (dt + 1) * 128]
                        for br in range(2):
                            nc.tensor.matmul(
                                ops[:, br * QCH:(br + 1) * QCH],
                                lhs,
                                fsbx[p][br][:],
                                start=(p == 0), stop=(p == 1),
                            )
                    osb = opool.tile([128, 2 * QCH], f32, tag="os", name="osb")
                    nc.vector.tensor_copy(osb[:], ops[:])
                    for br in range(2):
                        nc.sync.dma_start(
                            outs[br][dt * 128:(dt + 1) * 128,
                                     c_idx * QCH:(c_idx + 1) * QCH],
                            osb[:, br * QCH:(br + 1) * QCH],
                        )

            fsb_prev = None
            for c in range(NCH):
                fsb = [[None, None], [None, None]]  # [pair][branch]
                for p in range(2):
                    for br in range(2):
                        fsb[p][br] = fpool.tile([128, QCH], bf16, tag="fsb",
                                                bufs=8, name=f"fsb{p}{br}")
                for h in range(H):
                    j, par = h // 2, h % 2
                    # base-0 and base-64 views of this head's Q^T / K^T
                    k_lo = kt_pair[j] if par == 0 else kt_alt[j]
                    k_hi = kt_alt[j] if par == 0 else kt_pair[j]
                    q_lo = qt_pair[j] if par == 0 else qt_alt[j]
                    q_hi = qt_alt[j] if par == 0 else qt_pair[j]
                    ft = [ftp.tile([65, QCH], f32, tag="ft", name=f"ft{i}")
                          for i in range(2)]
                    for g in range(NG):
                        lg = lgp.tile([128, GW * QCH], f32, tag="lg")
                        for t2 in range(GW):
                            st = g * GW + t2
                            if st % 2 == 0:
                                nc.tensor.matmul(
                                    lg[:, t2 * QCH:(t2 + 1) * QCH],
                                    k_lo[0:64, st * 128:(st + 1) * 128],
                                    q_lo[0:64, c * QCH:(c + 1) * QCH],
                                    start=True, stop=True)
                            else:
                                nc.tensor.matmul(
                                    lg[:, t2 * QCH:(t2 + 1) * QCH],
                                    k_hi[64:128, st * 128:(st + 1) * 128],
                                    q_hi[64:128, c * QCH:(c + 1) * QCH],
                                    start=True, stop=True)
                        pw = [ppool.tile([128, GW * QCH], bf16, tag=f"pw{i}",
                                         name=f"pw{i}", bufs=3) for i in range(2)]
                        nc.scalar.activation(pw[0][:], lg[:], EXP, scale=-1.0)
                        nc.scalar.activation(pw[1][:], lg[:], EXP)
                        for t2 in range(GW):
                            st = g * GW + t2
                            vap = vap_for(st, h)
                            for br in range(2):
                                nc.tensor.matmul(
                                    ft[br][:],
                                    vap,
                                    pw[br][:, t2 * QCH:(t2 + 1) * QCH],
                                    start=(st == 0),
                                    stop=(st == SK - 1),
                                )
                        # previous chunk's output projection, one dout-tile
                        # per group during head 0 (spreads the PSUM-evac
                        # dependency so the in-order PE never waits on it)
                        if h == 0 and fsb_prev is not None:
                            emit_outproj(fsb_prev, c - 1, [g])
                    # normalize: feat / Z, Z = row 64.  Stage-major across
                    # branches so the DVE never idles waiting on the
                    # broadcast DMA mid-chain.  The Z row goes PSUM@p64 ->
                    # SBUF@p64 (plain copy; engines cannot shift partitions),
                    # broadcast-DMA (gpsimd queue) to [64,:]@p0, reciprocal,
                    # multiply.
                    ftc = [zpool.tile([65, QCH], f32, tag=f"ftc{i}", name=f"ftc{i}")
                           for i in range(2)]
                    zraw = [zpool.tile([64, QCH], f32, tag=f"zraw{i}", name=f"zraw{i}")
                            for i in range(2)]
                    zbt = [zpool.tile([64, QCH], f32, tag=f"zb{i}", name=f"zb{i}")
                           for i in range(2)]
                    for br in range(2):
                        nc.vector.tensor_copy(ftc[br][:], ft[br][:])
                        nc.gpsimd.dma_start(
                            zraw[br][:],
                            ftc[br][64:65, :].rearrange("p (o f) -> p o f", o=1)
                            .to_broadcast([1, 64, QCH]),
                        )
                    for br in range(2):
                        nc.vector.reciprocal_approx_fast(zbt[br][:], zraw[br][:])
                    for br in range(2):
                        if par == 0:
                            nc.vector.tensor_mul(
                                fsb[j][br][0:64, :], ftc[br][0:64, :], zbt[br][:])
                        else:
                            tmp = zpool.tile([64, QCH], bf16, tag=f"tmp{br}",
                                             name=f"tmp{br}")
                            nc.vector.tensor_mul(tmp[:], ftc[br][0:64, :], zbt[br][:])
                            nc.gpsimd.dma_start(fsb[j][br][64:128, :], tmp[:])
                fsb_prev = fsb
            emit_outproj(fsb_prev, NCH - 1, range(8))


def _build():
    if "nc" in _CACHE:
        return _CACHE["nc"]
    nc = bacc.Bacc("TRN2", target_bir_lowering=False, debug=False,
                   num_devices=N_CORES)
    with tile.TileContext(nc) as tc:
        _emit(tc)
    nc.compile()
    _CACHE["nc"] = nc
    return nc


def _prep_core_inputs(tokens, Wq, bq, Wk, bk, Wv, bv, Wo, bo):
    """Host-side marshaling: slice per core, transpose tokens, cast bf16."""
    scale = 1.0 / np.sqrt(HEAD_DIM)
    per_batch_tokT = [
        np.ascontiguousarray(tokens[b].T).astype(BF16) for b in range(B)
    ]
    in_maps = []
    for core in range(N_CORES):
        b, g = divmod(core, GROUPS)
        cols = slice(g * DH, (g + 1) * DH)
        # weights as [128, 8*DH]: din-tile t at column block t
        wq_s = (Wq[:, cols] * scale).astype(BF16).reshape(8, 128, DH)
        wq_s = np.ascontiguousarray(wq_s.transpose(1, 0, 2)).reshape(128, 8 * DH)
        wk_s = Wk[:, cols].astype(BF16).reshape(8, 128, DH)
        wk_s = np.ascontiguousarray(wk_s.transpose(1, 0, 2)).reshape(128, 8 * DH)
        wv_s = Wv[:, cols].astype(BF16).reshape(8, 128, DH)
        wv_s = np.ascontiguousarray(wv_s.transpose(1, 0, 2)).reshape(128, 8 * DH)
        # Wo rows for this group, pair p at column block p
        wo_s = Wo[cols, :].astype(BF16).reshape(2, 128, D)
        wo_s = np.ascontiguousarray(wo_s.transpose(1, 0, 2)).reshape(128, 2 * D)
        # biases: column j = q-pair j (rows 0:64 even head, 64:128 odd),
        # column 2+j = k-pair j
        bqk_s = np.zeros((128, 4), np.float32)
        for j in range(2):
            bqk_s[0:64, j] = bq[g * DH + (2 * j) * 64:g * DH + (2 * j + 1) * 64] * scale
            bqk_s[64:128, j] = bq[g * DH + (2 * j + 1) * 64:g * DH + (2 * j + 2) * 64] * scale
            bqk_s[0:64, 2 + j] = bk[g * DH + (2 * j) * 64:g * DH + (2 * j + 1) * 64]
            bqk_s[64:128, 2 + j] = bk[g * DH + (2 * j + 1) * 64:g * DH + (2 * j + 2) * 64]
        in_maps.append({
            "tokT": per_batch_tokT[b],
            "wq": wq_s, "wk": wk_s, "wv": wv_s, "wo": wo_s,
            "bqk": bqk_s,
        })
    return in_maps


def kernel(tokens, Wq, bq, Wk, bk, Wv, bv, Wo, bo):
    tokens = np.asarray(tokens, np.float32)
    Wq = np.asarray(Wq, np.float32); bq = np.asarray(bq, np.float32)
    Wk = np.asarray(Wk, np.float32); bk = np.asarray(bk, np.float32)
    Wv = np.asarray(Wv, np.float32); bv = np.asarray(bv, np.float32)
    Wo = np.asarray(Wo, np.float32); bo = np.asarray(bo, np.float32)

    nc = _build()
    in_maps = _prep_core_inputs(tokens, Wq, bq, Wk, bk, Wv, bv, Wo, bo)
    res = bass_utils.run_bass_kernel_spmd(
        nc, in_maps, core_ids=list(range(N_CORES)))
    _CACHE["last_result"] = res

    bo_eff = (bv.astype(np.float64) @ Wo.astype(np.float64)
              + bo.astype(np.float64)).astype(np.float32)

    out = []
    for name in ("outT_neg", "outT_pos"):
        full = np.empty((B, S, D), np.float32)
        for b in range(B):
            acc = res.results[b * GROUPS][name].copy()
            for g in range(1, GROUPS):
                acc += res.results[b * GROUPS + g][name]
            full[b] = acc.T
        full += bo_eff
        out.append(full)
    return tuple(out)


# revision 14
# speedup vs baseline: 1.1451x; 1.1415x over previous
"""Bidirectional attention (softmax(+logits) and softmax(-logits) branches)
on 8 Trainium2 NeuronCores.

Sharding: batch x head-group. Core c handles batch c//4 and heads
4*(c%4) .. 4*(c%4)+3. Each core computes its heads' Q/K/V projections,
both softmax branches, and a partial output projection (row-shard of Wo);
the host sums the 4 partials per batch and transposes.

All matmuls run in bf16 (fp32 matmul is 4x slower on the PE); PSUM
accumulation is fp32. The softmax uses unnormalized exp (logit range is
~N(0,1), no max-subtraction needed): P = exp(+/-logits) via wide ACT ops,
Z comes free from a ones-column appended to V, and the 1/Z normalization
uses reciprocal_approx_fast + a step-0-free-dim DMA broadcast.

PE throughput tricks (all HW-measured):
- head-pair col-tiled projections (even head -> psum[0:64], odd ->
  psum[64:128]) with 4 interleaved chunk accumulation chains;
- Q^T/K^T kept in both base-0 and base-64 copies so consecutive K=64
  logit matmuls alternate PE row-groups (~2x concurrency);
- featT/outproj accumulation chains interleaved in pairs sharing lhsT
  (single accumulation chains serialize fill/drain and are ~3x slower).

Host-side prep folds the 1/sqrt(d) scale into Wq, and bv@Wo+bo into a
host-side bias (exact because softmax rows sum to 1).
"""

import os
import sys

for _p in ("/opt/trn_rl_repo",):
    if _p not in sys.path:
        sys.path.insert(0, _p)

import numpy as np
import ml_dtypes

import concourse.bass as bass
import concourse.tile as tile
from concourse import bacc, mybir
from concourse import bass_utils

BF16 = ml_dtypes.bfloat16

B, S, D = 2, 2048, 1024
NUM_HEADS, HEAD_DIM = 16, 64
N_CORES = 8
GROUPS = 4                      # head groups (one per core within a batch)
H = NUM_HEADS // GROUPS         # heads per core = 4
DH = H * HEAD_DIM               # per-core head dims = 256
QCH = 512                       # q-chunk (matmul moving free dim)
NCH = S // QCH                  # 4 q-chunks
SK = S // 128                   # 16 s_k tiles
VSTRIDE = H * 65                # V' row stride: 4 heads x (64 + ones col)

f32 = mybir.dt.float32
bf16 = mybir.dt.bfloat16
EXP = mybir.ActivationFunctionType.Exp
IDENT = mybir.ActivationFunctionType.Identity

_CACHE = {}


def _emit(tc):
    nc = tc.nc
    tokT = nc.dram_tensor("tokT", [D, S], bf16, kind="ExternalInput").ap()
    wq = nc.dram_tensor("wq", [128, 8 * DH], bf16, kind="ExternalInput").ap()
    wk = nc.dram_tensor("wk", [128, 8 * DH], bf16, kind="ExternalInput").ap()
    wv = nc.dram_tensor("wv", [128, 8 * DH], bf16, kind="ExternalInput").ap()
    wo = nc.dram_tensor("wo", [128, 2 * D], bf16, kind="ExternalInput").ap()
    bqk = nc.dram_tensor("bqk", [128, 4], f32, kind="ExternalInput").ap()
    outs = [
        nc.dram_tensor("outT_neg", [D, S], f32, kind="ExternalOutput").ap(),
        nc.dram_tensor("outT_pos", [D, S], f32, kind="ExternalOutput").ap(),
    ]

    import contextlib

    with contextlib.ExitStack() as ctx:
        wp = ctx.enter_context(tc.tile_pool(name="wp", bufs=1))
        act = ctx.enter_context(tc.tile_pool(name="act", bufs=1))
        ppool = ctx.enter_context(tc.tile_pool(name="pp", bufs=2))
        fpool = ctx.enter_context(tc.tile_pool(name="fp", bufs=4))
        zpool = ctx.enter_context(tc.tile_pool(name="zp", bufs=4))
        opool = ctx.enter_context(tc.tile_pool(name="op", bufs=8))

        # ---- weight / bias / token loads -------------------------------
        wq_sb = wp.tile([128, 8 * DH], bf16, tag="wq")
        wk_sb = wp.tile([128, 8 * DH], bf16, tag="wk")
        wv_sb = wp.tile([128, 8 * DH], bf16, tag="wv")
        wo_sb = wp.tile([128, 2 * D], bf16, tag="wo")
        bqk_sb = wp.tile([128, 4], f32, tag="bqk")
        nc.sync.dma_start(wq_sb[:], wq)
        nc.sync.dma_start(wk_sb[:], wk)
        nc.sync.dma_start(wv_sb[:], wv)
        nc.sync.dma_start(wo_sb[:], wo)
        nc.sync.dma_start(bqk_sb[:], bqk)

        tok = []
        for t in range(8):
            tt = act.tile([128, S], bf16, tag=f"tok{t}")
            half = S // 2
            for hh in range(2):
                nc.sync.dma_start(tt[:, hh * half:(hh + 1) * half],
                                  tokT[t * 128:(t + 1) * 128, hh * half:(hh + 1) * half])
            tok.append(tt)

        # head-pair tiles: even head in partitions 0:64, odd in 64:128;
        # *_alt has the two halves swapped (so every head exists at both
        # partition bases -- lets logit matmuls alternate PE row groups)
        qt_pair = [act.tile([128, S], bf16, tag=f"qp{j}", name=f"qp{j}") for j in range(2)]
        kt_pair = [act.tile([128, S], bf16, tag=f"kp{j}", name=f"kp{j}") for j in range(2)]
        qt_alt = [act.tile([128, S], bf16, tag=f"qa{j}", name=f"qa{j}") for j in range(2)]
        kt_alt = [act.tile([128, S], bf16, tag=f"ka{j}", name=f"ka{j}") for j in range(2)]
        SKQ = SK // 4
        vp_q = [act.tile([128, SKQ * VSTRIDE], bf16, tag=f"vp{i}", name=f"vp{i}")
                for i in range(4)]

        def vap_for(st, h):
            q, r = divmod(st, SKQ)
            off = r * VSTRIDE + h * 65
            return vp_q[q][:, off:off + 65]

        # ---- phase 1 + 2: projections, attention, output projection ----
        # PSUM budget during attention: pj (2) + lg (2x2) + ft (2) = 8 banks.
        # pair-1 Q/K projections are emitted mid-attention (PE slack under
        # the ACT-bound exp stream), so the pj pool stays open throughout.
        GW = 2           # s_k-tiles per exp group (FD = GW*QCH = 1024)
        NG = SK // GW    # 8 groups per (head, chunk)
        with tc.tile_pool(name="qkp", bufs=1, space="PSUM") as qkp:
            def qk_proj(j):
                # head-pair col-tiled (even head -> psum[0:64], odd ->
                # [64:128]); 2-chunk interleaved accumulation chains, 2 passes
                for w_sb, pair, alt, bcol0 in (
                    (wq_sb, qt_pair, qt_alt, 0),
                    (wk_sb, kt_pair, kt_alt, 2),
                ):
                    for cp in range(2):          # chunk pairs
                        ps = [qkp.tile([128, QCH], f32, tag=f"pj{i}",
                                       name=f"pj{i}") for i in range(2)]
                        for t in range(8):
                            lo = t * DH + (2 * j) * 64
                            hi = t * DH + (2 * j + 1) * 64
                            for i in range(2):
                                cc = cp * 2 + i
                                rhs = tok[t][:, cc * QCH:(cc + 1) * QCH]
                                nc.tensor.matmul(ps[i][0:64, :], w_sb[:, lo:lo + 64],
                                                 rhs, start=(t == 0), stop=(t == 7))
                                nc.tensor.matmul(ps[i][64:128, :], w_sb[:, hi:hi + 64],
                                                 rhs, start=(t == 0), stop=(t == 7))
                        for i in range(2):
                            cc = cp * 2 + i
                            nc.scalar.activation(
                                pair[j][:, cc * QCH:(cc + 1) * QCH], ps[i][:],
                                IDENT, bias=bqk_sb[:, bcol0 + j:bcol0 + j + 1],
                            )
                    nc.sync.dma_start(alt[j][64:128, :], pair[j][0:64, :])
                    nc.sync.dma_start(alt[j][0:64, :], pair[j][64:128, :])

            with tc.tile_pool(name="pvp", bufs=1, space="PSUM") as pvp:
                qk_proj(0)
                # V: 4 interleaved s-tile chains, strided into V' quarters
                # (ones cols memset per quarter so featT deps stay local)
                for q in range(4):
                    ones_ap = vp_q[q].rearrange(
                        "p (s h x) -> p (s h) x", s=SKQ, h=H)[:, :, 64:65]
                    nc.gpsimd.memset(ones_ap, 1.0)
                for sp in range(SK // 2):
                    psv = [pvp.tile([128, DH], f32, tag=f"pv{i}", name=f"pv{i}")
                           for i in range(2)]
                    for t in range(8):
                        for i in range(2):
                            st = sp * 2 + i
                            nc.tensor.matmul(
                                psv[i][:],
                                tok[t][:, st * 128:(st + 1) * 128],
                                wv_sb[:, t * DH:(t + 1) * DH],
                                start=(t == 0), stop=(t == 7),
                            )
                    for i in range(2):
                        st = sp * 2 + i
                        q, r = divmod(st, SKQ)
                        dst = vp_q[q][:, r * VSTRIDE:(r + 1) * VSTRIDE]
                        dst3 = dst.rearrange("p (h x) -> p h x", h=H)[:, :, 0:64]
                        src3 = psv[i].rearrange("p (h x) -> p h x", h=H)
                        nc.vector.tensor_copy(dst3, src3)

            fsb_all = []
            with (
                tc.tile_pool(name="lgp", bufs=2, space="PSUM") as lgp,
                tc.tile_pool(name="ftp", bufs=2, space="PSUM") as ftp,
            ):
                for c in range(NCH):
                    fsb = [[None, None], [None, None]]  # [pair][branch]
                    for p in range(2):
                        for br in range(2):
                            fsb[p][br] = fpool.tile([128, QCH], bf16, tag="fsb",
                                                    bufs=16, name=f"fsb{p}{br}")
                    fsb_all.append(fsb)
                    for h in range(H):
                        j, par = h // 2, h % 2
                        if c == 0 and h == 2:
                            # pair-1 projections, in the ACT shadow of h0/h1
                            qk_proj(1)
                        # base-0 and base-64 views of this head's Q^T / K^T
                        k_lo = kt_pair[j] if par == 0 else kt_alt[j]
                        k_hi = kt_alt[j] if par == 0 else kt_pair[j]
                        q_lo = qt_pair[j] if par == 0 else qt_alt[j]
                        q_hi = qt_alt[j] if par == 0 else qt_pair[j]
                        ft = [ftp.tile([65, QCH], f32, tag="ft", name=f"ft{i}")
                              for i in range(2)]
                        for g in range(NG):
                            lg = lgp.tile([128, GW * QCH], f32, tag="lg")
                            for t2 in range(GW):
                                st = g * GW + t2
                                if st % 2 == 0:
                                    nc.tensor.matmul(
                                        lg[:, t2 * QCH:(t2 + 1) * QCH],
                                        k_lo[0:64, st * 128:(st + 1) * 128],
                                        q_lo[0:64, c * QCH:(c + 1) * QCH],
                                        start=True, stop=True)
                                else:
                                    nc.tensor.matmul(
                                        lg[:, t2 * QCH:(t2 + 1) * QCH],
                                        k_hi[64:128, st * 128:(st + 1) * 128],
                                        q_hi[64:128, c * QCH:(c + 1) * QCH],
                                        start=True, stop=True)
                            pw = [ppool.tile([128, GW * QCH], bf16, tag=f"pw{i}",
                                             name=f"pw{i}", bufs=3) for i in range(2)]
                            nc.scalar.activation(pw[0][:], lg[:], EXP, scale=-1.0)
                            nc.scalar.activation(pw[1][:], lg[:], EXP)
                            for t2 in range(GW):
                                st = g * GW + t2
                                vap = vap_for(st, h)
                                for br in range(2):
                                    nc.tensor.matmul(
                                        ft[br][:],
                                        vap,
                                        pw[br][:, t2 * QCH:(t2 + 1) * QCH],
                                        start=(st == 0),
                                        stop=(st == SK - 1),
                                    )
                        # normalize: feat / Z, Z = row 64.  Stage-major across
                        # branches; the whole-ft copy releases the PSUM bank
                        # early.  Z row goes PSUM@p64 -> SBUF@p64 (plain copy;
                        # engines cannot shift partitions), broadcast-DMA to
                        # [64,:]@p0, reciprocal, multiply.
                        ftc = [zpool.tile([65, QCH], f32, tag=f"ftc{i}",
                                          name=f"ftc{i}") for i in range(2)]
                        zraw = [zpool.tile([64, QCH], f32, tag=f"zraw{i}",
                                           name=f"zraw{i}") for i in range(2)]
                        zbt = [zpool.tile([64, QCH], f32, tag=f"zb{i}",
                                          name=f"zb{i}") for i in range(2)]
                        for br in range(2):
                            nc.vector.tensor_copy(ftc[br][:], ft[br][:])
                            nc.sync.dma_start(
                                zraw[br][:],
                                ftc[br][64:65, :].rearrange("p (o f) -> p o f", o=1)
                                .to_broadcast([1, 64, QCH]),
                            )
                        for br in range(2):
                            nc.vector.reciprocal_approx_fast(zbt[br][:], zraw[br][:])
                        for br in range(2):
                            if par == 0:
                                nc.vector.tensor_mul(
                                    fsb[j][br][0:64, :], ftc[br][0:64, :], zbt[br][:])
                            else:
                                tmp = zpool.tile([64, QCH], bf16, tag=f"tmp{br}",
                                                 name=f"tmp{br}")
                                nc.vector.tensor_mul(tmp[:], ftc[br][0:64, :],
                                                     zbt[br][:])
                                nc.sync.dma_start(fsb[j][br][64:128, :], tmp[:])

        # ---- phase 3: output projection (full PSUM available) ----------
        with tc.tile_pool(name="otp", bufs=4, space="PSUM") as otp:
            for c in range(NCH):
                for dt in range(8):
                    ops = otp.tile([128, 2 * QCH], f32, tag="ot", name="ops")
                    for p in range(2):
                        lhs = wo_sb[:, p * D + dt * 128:p * D + (dt + 1) * 128]
                        for br in range(2):
                            nc.tensor.matmul(
                                ops[:, br * QCH:(br + 1) * QCH],
                                lhs,
                                fsb_all[c][p][br][:],
                                start=(p == 0), stop=(p == 1),
                            )
                    osb = opool.tile([128, 2 * QCH], f32, tag="os", name="osb")
                    # alternate evacuation between DVE and ACT
                    if (c * 8 + dt) % 2 == 0:
                        nc.vector.tensor_copy(osb[:], ops[:])
                    else:
                        nc.scalar.copy(osb[:], ops[:])
                    for br in range(2):
                        nc.sync.dma_start(
                            outs[br][dt * 128:(dt + 1) * 128,
                                     c * QCH:(c + 1) * QCH],
                            osb[:, br * QCH:(br + 1) * QCH],
                        )


def _build():
    if "nc" in _CACHE:
        return _CACHE["nc"]
    nc = bacc.Bacc("TRN2", target_bir_lowering=False, debug=False,
                   num_devices=N_CORES)
    with tile.TileContext(nc) as tc:
        _emit(tc)
    nc.compile()
    _CACHE["nc"] = nc
    return nc


def _prep_core_inputs(tokens, Wq, bq, Wk, bk, Wv, bv, Wo, bo):
    """Host-side marshaling: slice per core, transpose tokens, cast bf16."""
    scale = 1.0 / np.sqrt(HEAD_DIM)
    per_batch_tokT = [
        np.ascontiguousarray(tokens[b].T).astype(BF16) for b in range(B)
    ]
    in_maps = []
    for core in range(N_CORES):
        b, g = divmod(core, GROUPS)
        cols = slice(g * DH, (g + 1) * DH)
        # weights as [128, 8*DH]: din-tile t at column block t
        wq_s = (Wq[:, cols] * scale).astype(BF16).reshape(8, 128, DH)
        wq_s = np.ascontiguousarray(wq_s.transpose(1, 0, 2)).reshape(128, 8 * DH)
        wk_s = Wk[:, cols].astype(BF16).reshape(8, 128, DH)
        wk_s = np.ascontiguousarray(wk_s.transpose(1, 0, 2)).reshape(128, 8 * DH)
        wv_s = Wv[:, cols].astype(BF16).reshape(8, 128, DH)
        wv_s = np.ascontiguousarray(wv_s.transpose(1, 0, 2)).reshape(128, 8 * DH)
        # Wo rows for this group, pair p at column block p
        wo_s = Wo[cols, :].astype(BF16).reshape(2, 128, D)
        wo_s = np.ascontiguousarray(wo_s.transpose(1, 0, 2)).reshape(128, 2 * D)
        # biases: column j = q-pair j (rows 0:64 even head, 64:128 odd),
        # column 2+j = k-pair j
        bqk_s = np.zeros((128, 4), np.float32)
        for j in range(2):
            bqk_s[0:64, j] = bq[g * DH + (2 * j) * 64:g * DH + (2 * j + 1) * 64] * scale
            bqk_s[64:128, j] = bq[g * DH + (2 * j + 1) * 64:g * DH + (2 * j + 2) * 64] * scale
            bqk_s[0:64, 2 + j] = bk[g * DH + (2 * j) * 64:g * DH + (2 * j + 1) * 64]
            bqk_s[64:128, 2 + j] = bk[g * DH + (2 * j + 1) * 64:g * DH + (2 * j + 2) * 64]
        in_maps.append({
            "tokT": per_batch_tokT[b],
            "wq": wq_s, "wk": wk_s, "wv": wv_s, "wo": wo_s,
            "bqk": bqk_s,
        })
    return in_maps


def kernel(tokens, Wq, bq, Wk, bk, Wv, bv, Wo, bo):
    tokens = np.asarray(tokens, np.float32)
    Wq = np.asarray(Wq, np.float32); bq = np.asarray(bq, np.float32)
    Wk = np.asarray(Wk, np.float32); bk = np.asarray(bk, np.float32)
    Wv = np.asarray(Wv, np.float32); bv = np.asarray(bv, np.float32)
    Wo = np.asarray(Wo, np.float32); bo = np.asarray(bo, np.float32)

    nc = _build()
    in_maps = _prep_core_inputs(tokens, Wq, bq, Wk, bk, Wv, bv, Wo, bo)
    res = bass_utils.run_bass_kernel_spmd(
        nc, in_maps, core_ids=list(range(N_CORES)))
    _CACHE["last_result"] = res

    bo_eff = (bv.astype(np.float64) @ Wo.astype(np.float64)
              + bo.astype(np.float64)).astype(np.float32)

    out = []
    for name in ("outT_neg", "outT_pos"):
        full = np.empty((B, S, D), np.float32)
        for b in range(B):
            acc = res.results[b * GROUPS][name].copy()
            for g in range(1, GROUPS):
                acc += res.results[b * GROUPS + g][name]
            full[b] = acc.T
        full += bo_eff
        out.append(full)
    return tuple(out)
